# revision 59
# baseline (speedup 1.0000x reference)
"""MoE adapter layer kernel for Trainium2 (8 NeuronCores, data-parallel over batch).

Per-core plan (core b handles batch element b of 8):
  - x[b] arrives transposed+pre-cast as xb [H=1024, L=2048] bf16 plus a bf16
    error-compensation tensor xe = bf16(x - fp32(xb)) so the residual add is
    fp32-exact to ~1e-5 while only bf16 bytes move for x.
  - Device row-sums xb (VectorE) for the mean-pool, computes router logits via
    PE (pooled @ Wr.T / L folded into a host-scaled WrT), softmax + top-k
    extraction with plain DVE ops (reduce_max / is_ge / iota dot).
  - Top-k expert indices are read into SP registers; Wd/Wu/bd/bu of ONLY the
    selected experts are fetched via dynamic (register-offset) DMA.
  - Down-proj h^T = relu(Wd^T-chunks @ xb + bd) with the gate value folded in
    (g*relu(z+bd) == relu(g*z+g*bd) since g>0), up-proj accumulates all
    selected experts into PSUM, then xsum = xb+xe (fp32) and one
    scalar_tensor_tensor adds the gate-weighted bu and the residual.
  - Output written as yT [1024, 2048] fp32; host transposes back.

Everything except transpose/cast/shard prep happens on device.
"""

import os
import sys

import numpy as np

for _p in ("/opt/trn_rl_repo", "/root/.axon_site/_ro/trn_rl_repo"):
    if os.path.isdir(_p) and _p not in sys.path:
        sys.path.append(_p)

import ml_dtypes  # noqa: E402

B, L, H, E, R = 8, 2048, 1024, 8, 256
HC = H // 128          # 8 H-chunks
RC = R // 128          # 2 R-chunks
TBLK = 512             # token block (matmul moving free dim)
NT = L // TBLK         # 4 token blocks

_BF16 = ml_dtypes.bfloat16

_cache = {}


def _fix_sync_waits(nc, mybir, max_waits=1):
    """walrus in this toolchain only encodes 1 sync wait per instruction;
    split excess waits into preceding same-engine NoOps."""
    ctr = 0
    for b in nc.main_func.blocks:
        insts = b.instructions
        new = []
        changed = False
        for ins in insts:
            si = ins.sync_info
            if si is not None and len(si.on_wait) > max_waits:
                waits = list(si.on_wait)
                excess, keep = waits[:-max_waits], waits[-max_waits:]
                for i in range(0, len(excess), max_waits):
                    ctr += 1
                    nop = mybir.InstNoOp(
                        name=f"waitsplit_nop_{ctr}_{ins.name}", engine=ins.engine
                    )
                    nop.sync_info = mybir.SyncInfo(
                        on_wait=excess[i : i + max_waits], on_update=[]
                    )
                    new.append(nop)
                ins.sync_info = mybir.SyncInfo(
                    on_wait=keep, on_update=list(si.on_update)
                )
                changed = True
            new.append(ins)
        if changed:
            b.instructions = new


def _build(top_k, repeat=1):
    import concourse.bass as bass
    import concourse.mybir as mybir
    from concourse.tile import TileContext

    fp32 = mybir.dt.float32
    bf16 = mybir.dt.bfloat16
    int32 = mybir.dt.int32
    K = top_k

    nc = bass.Bass("TRN2", target_bir_lowering=False, debug=False, num_devices=8)

    xb_d = nc.declare_dram_parameter("xb", [HC, 128, L], bf16, isOutput=False)
    xe_d = nc.declare_dram_parameter("xe", [HC, 128, L], bf16, isOutput=False)
    WdC = nc.declare_dram_parameter("WdC", [E, RC, HC, 128, 128], bf16, isOutput=False)
    WuC = nc.declare_dram_parameter("WuC", [E, RC, 128, H], bf16, isOutput=False)
    bd_r = nc.declare_dram_parameter("bd_r", [E, RC, 128], fp32, isOutput=False)
    bu_r = nc.declare_dram_parameter("bu_r", [E, HC, 128], fp32, isOutput=False)
    WrT = nc.declare_dram_parameter("WrT", [HC, 128, E], fp32, isOutput=False)
    iota = nc.declare_dram_parameter("iota", [1, E], fp32, isOutput=False)
    yT = nc.declare_dram_parameter("yT", [HC, 128, L], fp32, isOutput=True)

    SP = mybir.EngineType.SP
    Relu = mybir.ActivationFunctionType.Relu
    Exp = mybir.ActivationFunctionType.Exp
    Copy = mybir.ActivationFunctionType.Copy
    add = mybir.AluOpType.add
    mult = mybir.AluOpType.mult
    is_ge = mybir.AluOpType.is_ge

    with TileContext(nc) as tc:
        with (
            tc.tile_pool(name="xb", bufs=1) as xb_pool,
            tc.tile_pool(name="xe", bufs=1) as xe_pool,
            tc.tile_pool(name="w", bufs=1) as w_pool,
            tc.tile_pool(name="small", bufs=1) as sm_pool,
            tc.tile_pool(name="h", bufs=2) as h_pool,
            tc.tile_pool(name="xs", bufs=1) as xs_pool,
            tc.tile_pool(name="y", bufs=6) as y_pool,
            tc.tile_pool(name="ph", bufs=3, space="PSUM") as ph_pool,
            tc.tile_pool(name="py", bufs=4, space="PSUM") as py_pool,
            tc.tile_pool(name="pr", bufs=1, space="PSUM") as pr_pool,
        ):
            def emit():
                # ---- stream xb in (bf16) as half-chunks; pool as they land ----
                xb_sb = []
                NHALF = 2
                HLEN = L // NHALF
                QLEN = HLEN // 2
                # pieces: 15 halves + 2 quarters at the very end (the final
                # piece's pooling latency sits on the router critical path)
                pieces = []
                for c in range(HC):
                    for hh in range(NHALF):
                        if c == HC - 1 and hh == NHALF - 1:
                            pieces.append((c, hh * HLEN, QLEN))
                            pieces.append((c, hh * HLEN + QLEN, QLEN))
                        else:
                            pieces.append((c, hh * HLEN, HLEN))
                pooled2 = sm_pool.tile([128, len(pieces)], fp32, tag="pooled2")
                for c in range(HC):
                    t = xb_pool.tile([128, L], bf16, tag=f"xb{c}")
                    xb_sb.append(t)
                for c, off, ln in pieces:
                    nc.sync.dma_start(
                        out=xb_sb[c][:, off : off + ln],
                        in_=xb_d[c][:, off : off + ln],
                    )
                wrt_sb = sm_pool.tile([128, HC, E], fp32, tag="wrt")
                nc.sync.dma_start(
                    out=wrt_sb[:], in_=WrT.rearrange("c p e -> p c e")
                )
                iota_sb = sm_pool.tile([1, E], fp32, tag="iota")
                nc.sync.dma_start(out=iota_sb[:], in_=iota[:])
                ones_sb = sm_pool.tile([1, 128], fp32, tag="ones")
                nc.vector.memset(ones_sb[:], 1.0)

                # pooling split between DVE (tensor_reduce) and ACT (exact
                # fp32 accum of the bf16 values via Copy+accum_out)
                for i, (c, off, ln) in enumerate(pieces):
                    hs = slice(off, off + ln)
                    if i % 2 == 0:
                        nc.vector.tensor_reduce(
                            pooled2[:, i : i + 1], xb_sb[c][:, hs],
                            mybir.AxisListType.X, add,
                        )
                    else:
                        scr = sm_pool.tile(
                            [128, ln], bf16, tag=f"pscr{i % 4}"
                        )
                        nc.scalar.activation(
                            scr[:], xb_sb[c][:, hs], Copy,
                            accum_out=pooled2[:, i : i + 1],
                        )

                # ---- router: logits[1, E] = pooled.T @ (Wr.T / L) ----
                logits_ps = pr_pool.tile([1, E], fp32, tag="logits")
                for i, (c, off, ln) in enumerate(pieces):
                    nc.tensor.matmul(
                        logits_ps[:],
                        pooled2[:, i : i + 1],
                        wrt_sb[:, c, :],
                        start=(i == 0),
                        stop=(i == len(pieces) - 1),
                    )

                # ---- top-k selection directly on logits (pre-softmax) ----
                logits = sm_pool.tile([1, E], fp32, tag="logits_sb")
                nc.vector.tensor_copy(logits[:], logits_ps[:])
                idx_i32 = sm_pool.tile([1, K], int32, tag="idxi")
                masks = []
                lmax = []
                lz = logits
                for k in range(K):
                    mk = sm_pool.tile([1, 1], fp32, tag=f"m{k}")
                    nc.vector.tensor_reduce(
                        mk[:], lz[:], mybir.AxisListType.X, mybir.AluOpType.max
                    )
                    lmax.append(mk)
                    # fused (lz >= mk) * iota in one op -- the index chain
                    # is critical; the pure mask (for softmax g12 and the
                    # lz update) is recomputed off this chain just below
                    mi = sm_pool.tile([1, E], fp32, tag=f"mi{k}")
                    nc.vector.scalar_tensor_tensor(
                        mi[:], lz[:], mk[0:1, 0:1], iota_sb[:], is_ge, mult
                    )
                    # reduce writes the int32 index directly (DVE converts on
                    # write) -- no separate cast op on the critical chain.
                    # Values are exact small integers, so int32 accumulation
                    # is lossless here.
                    with nc.allow_low_precision(reason="exact small-int index"):
                        nc.vector.tensor_reduce(
                            idx_i32[0:1, k : k + 1], mi[:],
                            mybir.AxisListType.X, add,
                        )
                    maskk = sm_pool.tile([1, E], fp32, tag=f"mask{k}")
                    nc.vector.tensor_scalar(
                        maskk[:], lz[:], mk[0:1, 0:1], None, is_ge
                    )
                    masks.append(maskk)
                    if k + 1 < K:
                        lznext = sm_pool.tile([1, E], fp32, tag=f"lz{k}")
                        # lznext = logits - BIG*mask  (mask out extracted max)
                        nc.vector.scalar_tensor_tensor(
                            lznext[:], maskk[:], -1.0e30, lz[:], mult, add
                        )
                        lz = lznext

                # ---- fetch selected experts' weights via dynamic DMA.
                # Expert slot 0 loads its index + weights via SP, slot 1 via
                # ACT: the two SBUF->register loads (~1.6us each) and the DMA
                # dispatches run in parallel. wd is rc-split so the first
                # matmul group can start after only 256KB lands. ----
                ld_eng = [nc.sync, nc.scalar]
                ld_engt = [[SP], [mybir.EngineType.Activation]]
                eks = []
                wd_sb = []  # wd_sb[k][rc] tiles [128, HC, 128]
                wu_sb = []
                bd_sb, bu_sb = [], []
                for k in range(K):
                    ek = nc.values_load(
                        idx_i32[0:1, k : k + 1],
                        engines=ld_engt[k % 2],
                        min_val=0,
                        max_val=E - 1,
                        skip_runtime_bounds_check=True,
                    )
                    eks.append(ek)
                    wdk = []
                    for rc in range(RC):
                        wd = w_pool.tile([128, HC, 128], bf16, tag=f"wd{k}_{rc}")
                        if k == 0 and rc == 0:
                            # split: first hc-half lands 0.36us earlier and
                            # feeds the first 4 matmuls of the first group
                            ld_eng[k % 2].dma_start(
                                out=wd[:, 0 : HC // 2, :],
                                in_=WdC[bass.ds(ek, 1), rc, 0 : HC // 2].rearrange(
                                    "o c p r -> (o p) c r"
                                ),
                            )
                            ld_eng[k % 2].dma_start(
                                out=wd[:, HC // 2 :, :],
                                in_=WdC[bass.ds(ek, 1), rc, HC // 2 :].rearrange(
                                    "o c p r -> (o p) c r"
                                ),
                            )
                        else:
                            ld_eng[k % 2].dma_start(
                                out=wd[:],
                                in_=WdC[bass.ds(ek, 1), rc].rearrange(
                                    "o c p r -> (o p) c r"
                                ),
                            )
                        wdk.append(wd)
                    wd_sb.append(wdk)
                    # tiny bias loads between wd (PE-critical) and wu
                    # (up-phase): they unblock the gbd/gbu -> relu chain
                    bdk = sm_pool.tile([128, RC], fp32, tag=f"bd{k}")
                    ld_eng[k % 2].dma_start(
                        out=bdk[:],
                        in_=bd_r[bass.ds(ek, 1)].rearrange("o c p -> (o p) c"),
                    )
                    bd_sb.append(bdk)
                    buk = sm_pool.tile([128, HC], fp32, tag=f"bu{k}")
                    ld_eng[k % 2].dma_start(
                        out=buk[:],
                        in_=bu_r[bass.ds(ek, 1)].rearrange("o c p -> (o p) c"),
                    )
                    bu_sb.append(buk)
                for k in range(K):
                    wu = w_pool.tile([128, RC, H], bf16, tag=f"wu{k}")
                    ld_eng[k % 2].dma_start(
                        out=wu[:],
                        in_=WuC[bass.ds(eks[k], 1)].rearrange("o c p h -> (o p) c h"),
                    )
                    wu_sb.append(wu)

                # ---- xe full rows on Pool (SWDGE), gated behind the router
                # by a dummy copy so they never steal prefix DMA bandwidth ----
                xe_sb = []
                gate_src = wu_sb[K - 1].bitcast(int32)
                for c in range(HC):
                    t = xe_pool.tile([128, L], bf16, tag=f"xe{c}")
                    nc.gpsimd.tensor_copy(
                        t.bitcast(int32)[0:1, 0:1], gate_src[0:1, 0:1, 0:1]
                    )
                    nc.gpsimd.dma_start(out=t[:], in_=xe_d[c])
                    xe_sb.append(t)

                # ---- softmax values for the selected experts (off critical path)
                # g_k = exp(l_k - l_max) / sum_j exp(l_j - l_max); l_max = lmax[0]
                negmax = sm_pool.tile([1, 1], fp32, tag="negmax")
                nc.vector.tensor_scalar_mul(negmax[:], lmax[0][:], -1.0)
                expv = sm_pool.tile([1, E], fp32, tag="expv")
                nc.scalar.activation(
                    expv[:], logits[:], Exp, bias=negmax[0:1, 0:1], scale=1.0
                )
                esum = sm_pool.tile([1, 1], fp32, tag="esum")
                nc.vector.tensor_reduce(
                    esum[:], expv[:], mybir.AxisListType.X, add
                )
                rsum = sm_pool.tile([1, 1], fp32, tag="rsum")
                nc.vector.reciprocal(rsum[:], esum[:])
                g12 = sm_pool.tile([1, K], fp32, tag="g12")
                for k in range(K):
                    ge_t = sm_pool.tile([1, E], fp32, tag=f"ge{k}")
                    nc.vector.tensor_tensor(ge_t[:], expv[:], masks[k][:], mult)
                    gs_t = sm_pool.tile([1, 1], fp32, tag=f"gs{k}")
                    nc.vector.tensor_reduce(
                        gs_t[:], ge_t[:], mybir.AxisListType.X, add
                    )
                    nc.vector.tensor_tensor(
                        g12[0:1, k : k + 1], gs_t[:], rsum[:], mult
                    )

                # ---- PE warm-up: HAM drops the PE clock to 1.2GHz after
                # ~3.4us idle; dummy matmuls bridge the topk/weight-fetch gap
                # so the main loop starts at full clock ----
                warm_ps = pr_pool.tile([128, TBLK], fp32, tag="logits")
                for _ in range(11):
                    nc.tensor.matmul(
                        warm_ps[:], xb_sb[0][:, 0:128], xb_sb[0][:, 0:TBLK],
                        start=True, stop=True,
                    )

                # gate broadcast + gate-scaled biases are emitted lazily
                # (inside the main loop, after the first down group) so the
                # K=1 matmul does not block the in-order PE queue.
                g_ps = pr_pool.tile([128, K], fp32, tag="logits")
                g_sb = sm_pool.tile([128, K], fp32, tag="gsb")
                gbd_sb = []
                gbu_sb = sm_pool.tile([128, HC], fp32, tag="gbu")

                def emit_bias():
                    for k in range(K):
                        gbd = sm_pool.tile([128, RC], fp32, tag=f"gbd{k}")
                        nc.vector.tensor_scalar_mul(
                            gbd[:], bd_sb[k][:], g_sb[:, k : k + 1]
                        )
                        gbd_sb.append(gbd)
                        if k == 0:
                            nc.vector.tensor_scalar_mul(
                                gbu_sb[:], bu_sb[0][:], g_sb[:, 0:1]
                            )
                        else:
                            nc.vector.scalar_tensor_tensor(
                                gbu_sb[:], bu_sb[k][:], g_sb[:, k : k + 1],
                                gbu_sb[:], mult, add,
                            )

                # ---- main compute: per token block, down-proj then up-proj ----
                first_down_done = False
                # token blocks: full-width for the steady state, two half
                # blocks at the end so the drain tail (STT + out-DMA of the
                # final block) is half as long.
                BLOCKS = [(i * TBLK, TBLK) for i in range(NT)]

                def emit_xs(t):
                    off, w = BLOCKS[t]
                    ts = slice(off, off + w)
                    xs_blk = []
                    for c in range(HC):
                        xs = xs_pool.tile([128, w], fp32, tag=f"xs{t % 2}{c}")
                        nc.gpsimd.tensor_tensor(
                            xs[:], xb_sb[c][:, ts], xe_sb[c][:, ts], add
                        )
                        xs_blk.append(xs)
                    return xs_blk

                def emit_down(t):
                    nonlocal first_down_done
                    off, w = BLOCKS[t]
                    ts = slice(off, off + w)
                    h_sb = [[None] * RC for _ in range(K)]
                    for rc in range(RC):
                        for k in range(K):
                            psum_h = ph_pool.tile([128, w], fp32, tag="psh")
                            for c in range(HC):
                                nc.tensor.matmul(
                                    psum_h[:],
                                    wd_sb[k][rc][:, c, :],
                                    xb_sb[c][:, ts],
                                    start=(c == 0),
                                    stop=(c == HC - 1),
                                )
                            if not first_down_done:
                                # PE is in-order: emit the tiny gate-broadcast
                                # matmul only after the first (weight-gated)
                                # group so it never blocks the queue.
                                nc.tensor.matmul(
                                    g_ps[:], ones_sb[:], g12[:],
                                    start=True, stop=True,
                                )
                                nc.scalar.activation(g_sb[:], g_ps[:], Copy)
                                emit_bias()
                                first_down_done = True
                            ht = h_pool.tile([128, w], bf16, tag=f"h{t % 2}{k}{rc}")
                            # g*relu(z + bd) == relu(g*z + g*bd), g > 0
                            nc.scalar.activation(
                                ht[:], psum_h[:], Relu,
                                bias=gbd_sb[k][:, rc : rc + 1],
                                scale=g_sb[:, k : k + 1],
                            )
                            h_sb[k][rc] = ht
                    return h_sb

                def emit_up(t, h_sb, xs_blk):
                    off, w = BLOCKS[t]
                    ts = slice(off, off + w)
                    for c in range(HC):
                        psum_y = py_pool.tile([128, w], fp32, tag="psy")
                        first = True
                        for k in range(K):
                            for rc in range(RC):
                                nc.tensor.matmul(
                                    psum_y[:],
                                    wu_sb[k][:, rc, c * 128 : (c + 1) * 128],
                                    h_sb[k][rc][:],
                                    start=first,
                                    stop=(k == K - 1 and rc == RC - 1),
                                )
                                first = False
                        yt = y_pool.tile([128, w], fp32, tag="yt")
                        # yt = (psum_y + gbu) + xsum
                        nc.vector.scalar_tensor_tensor(
                            yt[:], psum_y[:], gbu_sb[:, c : c + 1],
                            xs_blk[c][:], add, add,
                        )
                        nc.sync.dma_start(out=yT[c][:, ts], in_=yt[:])

                # 1-deep software pipeline: down(t+1) is emitted before up(t)
                # so PE has independent work while ACT finishes relu(t).
                NB = len(BLOCKS)
                prev = None
                xs_prev = None
                for t in range(NB):
                    h_cur = emit_down(t)
                    xs_cur = emit_xs(t) if t == 0 else None
                    if prev is not None:
                        emit_up(t - 1, prev, xs_prev)
                    if t > 0:
                        xs_cur = emit_xs(t)
                    prev, xs_prev = h_cur, xs_cur
                emit_up(NB - 1, prev, xs_prev)

            for _rep in range(repeat):
                emit()

    _fix_sync_waits(nc, mybir)
    return nc


def _get_nc(top_k, repeat=1):
    key = (top_k, repeat)
    if key not in _cache:
        _cache[key] = _build(top_k, repeat)
    return _cache[key]


def _prep_shared(Wd, bd, Wu, bu, Wr):
    WdC = np.ascontiguousarray(
        Wd.astype(_BF16).reshape(E, HC, 128, RC, 128).transpose(0, 3, 1, 2, 4)
    )
    WuC = np.ascontiguousarray(Wu.astype(_BF16)).reshape(E, RC, 128, H)
    bd_rv = np.ascontiguousarray(bd.astype(np.float32)).reshape(E, RC, 128)
    bu_rv = np.ascontiguousarray(bu.astype(np.float32)).reshape(E, HC, 128)
    WrTv = np.ascontiguousarray(Wr.astype(np.float32).T / np.float32(L)).reshape(
        HC, 128, E
    )
    iotav = np.arange(E, dtype=np.float32).reshape(1, E)
    return WdC, WuC, bd_rv, bu_rv, WrTv, iotav


def _prep_core(xb_full):
    """xb_full: x[b] fp32 [L, H] -> (xb, xe) bf16 [HC, 128, L]."""
    xT = np.ascontiguousarray(xb_full.astype(np.float32).T)  # [H, L]
    xbT = xT.astype(_BF16)
    xeT = (xT - xbT.astype(np.float32)).astype(_BF16)
    return xbT.reshape(HC, 128, L), xeT.reshape(HC, 128, L)


def build_in_maps(x, Wd, bd, Wu, bu, Wr):
    WdC, WuC, bd_rv, bu_rv, WrTv, iotav = _prep_shared(Wd, bd, Wu, bu, Wr)
    in_maps = []
    for b in range(B):
        xbT, xeT = _prep_core(x[b])
        in_maps.append(
            {
                "xb": xbT,
                "xe": xeT,
                "WdC": WdC,
                "WuC": WuC,
                "bd_r": bd_rv,
                "bu_r": bu_rv,
                "WrT": WrTv,
                "iota": iotav,
            }
        )
    return in_maps


def kernel(x, Wd, bd, Wu, bu, Wr, top_k):
    from concourse.bass_utils import run_bass_kernel_spmd

    x = np.asarray(x)
    k = min(int(top_k), E)
    nc = _get_nc(k)
    in_maps = build_in_maps(
        x, np.asarray(Wd), np.asarray(bd), np.asarray(Wu), np.asarray(bu),
        np.asarray(Wr),
    )
    res = run_bass_kernel_spmd(nc, in_maps, list(range(B)))

    out = np.empty((B, L, H), dtype=np.float32)
    for b in range(B):
        out[b] = res.results[b]["yT"].reshape(H, L).T
    return out
